# revision 1
# baseline (speedup 1.0000x reference)
"""DETR loss (cost matrix + Hungarian matching + losses) on 8 Trainium2 cores.

Sharding: data-parallel over batch. Each core handles 4 images as 2 pairs of 2
images packed into 128 SBUF partitions (2 images x 64 targets). The device
computes, per image, the [T=64, Q=300] matching-cost block (L1 cdist + class
cost + pairwise GIoU cost). The inherently serial Hungarian assignment runs on
host (exactly as in the reference, whose matcher is host-side numpy), and the
scalar loss is assembled on host from the matched pairs.
"""
import numpy as np

B, Q, T, C = 32, 300, 64, 2
N_CORES = 8
IMGS_PER_CORE = B // N_CORES          # 4
PAIRS_PER_CORE = IMGS_PER_CORE // 2   # 2
CLS_SCALE = 0.1
BBOX_SCALE = 5.0
GIOU_SCALE = 2.0

PIPE_DT = "bfloat16"   # dtype of the post-PSUM cost pipeline

# engine assignment knobs (tuned via CoreSim cost model)
R_ENGS = ["scalar", "scalar", "vector", "vector"]   # r1..r4
B_ENGS = ["scalar", "scalar", "scalar", "scalar"]   # b1..b4
TT_ENG = "gpsimd"    # LB / P2 / OUT adds
WE_ENG = "vector"

_CACHE = {}


def _split_wide_waits(nc, mybir, max_waits=1):
    """This walrus rejects instructions carrying >1 sem-wait; hoist extra
    waits onto NoOp carriers inserted just before (same engine, in-order)."""
    n_new = 0
    for bb in nc.main_func.blocks:
        insts = bb.instructions
        i = 0
        while i < len(insts):
            ins = insts[i]
            si = ins.sync_info
            if (
                si is not None
                and si.on_wait is not None
                and len(si.on_wait) > max_waits
            ):
                waits = list(si.on_wait)
                si.on_wait = waits[:max_waits]
                extra = waits[max_waits:]
                for j in range(0, len(extra), max_waits):
                    nd = mybir.InstNoOp(name=f"{ins.name}-xw{n_new}", ins=[], outs=[])
                    nd.engine = ins.engine
                    nd.sync_info = mybir.SyncInfo(
                        on_wait=extra[j : j + max_waits], on_update=[]
                    )
                    nc.register_instruction(nd, overwrite=True)
                    insts.insert(i, nd)
                    n_new += 1
                    i += 1
            i += 1
    return n_new


def _build_program():
    import concourse.bass as bass
    import concourse.mybir as mybir
    from concourse.tile import TileContext

    f32 = mybir.dt.float32
    DT = getattr(mybir.dt, PIPE_DT)
    op = mybir.AluOpType
    AF = mybir.ActivationFunctionType
    # qrows slots: px1, -px2, py1, -py2 | pcx, pcy, pw, ph | area1, f
    NQROW = 10
    # trows: ty1, nty2, tw, th, area2, ntx1, tx2, ntcx, ntcy, ntw, nth
    NTROW = 11
    QW3 = NQROW * Q

    bf16 = mybir.dt.bfloat16
    nc = bass.Bass()
    # per pair: 3 groups x 2 imgs x 4 quantity-slots of Q cols
    qrows = nc.declare_dram_parameter("qrows", [PAIRS_PER_CORE, 96, 4 * Q], bf16, isOutput=False)
    trows = nc.declare_dram_parameter("trows", [128, PAIRS_PER_CORE * NTROW], f32, isOutput=False)
    cost_o = nc.declare_dram_parameter("cost", [PAIRS_PER_CORE, 128, Q], DT, isOutput=True)

    with TileContext(nc) as tc:
        with (
            nc.allow_low_precision(reason="bf16 cost pipeline; assignment-tolerant"),
            tc.tile_pool(name="const", bufs=1) as cpool,
            tc.tile_pool(name="sb", bufs=2) as sb,
            tc.tile_pool(name="ps", bufs=4, space="PSUM") as ps,
        ):
            # indicator built on-chip at each legal matmul base (0/32/64):
            # row0 = [1]*64+[0]*64 (applied to A-B), row1 = all ones (applied to B)
            indt = cpool.tile([96, 128], bf16)
            for g in range(3):
                nc.vector.memset(indt[g * 32:g * 32 + 2, :], 1.0)
                nc.vector.memset(indt[g * 32:g * 32 + 1, 64:128], 0.0)
            # warm the ACT table set (Relu+Abs) while input DMAs are in flight
            warm = cpool.tile([2, 128], DT)
            nc.scalar.activation(warm[:], indt[0:2, :], AF.Relu)
            nc.scalar.activation(warm[:], indt[0:2, :], AF.Abs)

            # per-pair input DMA into partition groups at legal matmul bases
            qts = []
            for p in range(PAIRS_PER_CORE):
                qt = sb.tile([96, 4 * Q], bf16, tag=f"qt{p}")
                (nc.sync if p % 2 == 0 else nc.gpsimd).dma_start(out=qt[:], in_=qrows[p])
                qts.append(qt)
            trt = sb.tile([128, PAIRS_PER_CORE * NTROW], f32, tag="trt")
            nc.scalar.dma_start(out=trt[:], in_=trows[:])

            def mm_round(p, ks):
                Mr = ps.tile([128, 2 * 512], f32, tag="mega")
                Mrv = Mr[:].rearrange("p (s k) -> p s k", k=512)
                for i, k in enumerate(ks):
                    g, ck = (0, k) if k < 4 else ((1, k - 4) if k < 7 else (2, k - 7))
                    nc.tensor.matmul(Mrv[:, i, 0:Q], lhsT=indt[g * 32:g * 32 + 2, :],
                                     rhs=qts[p][g * 32:g * 32 + 2, ck * Q:(ck + 1) * Q],
                                     start=True, stop=True)
                return Mrv

            def fused(out_ap, psum_ap, bias_ap, kind, eng):
                if eng == "scalar":
                    nc.scalar.activation(out_ap, psum_ap,
                                         AF.Relu if kind == "relu" else AF.Abs,
                                         bias=bias_ap)
                else:
                    getattr(nc, eng).tensor_scalar(
                        out=out_ap, in0=psum_ap, scalar1=bias_ap, scalar2=0.0,
                        op0=op.add,
                        op1=op.max if kind == "relu" else op.abs_max)

            st = [dict() for _ in range(PAIRS_PER_CORE)]
            for p in range(PAIRS_PER_CORE):
                st[p]["Mx"] = mm_round(p, [0, 1])      # px1, -px2
                st[p]["My"] = mm_round(p, [2, 3])      # py1, -py2
            for p in range(PAIRS_PER_CORE):
                def sc(k, p=p):
                    return trt[:, p * NTROW + k:p * NTROW + k + 1]
                R13 = sb.tile([128, 2 * Q], DT, tag=f"R13_{p}")
                R24 = sb.tile([128, 2 * Q], DT, tag=f"R24_{p}")
                fused(R13[:, :Q], st[p]["Mx"][:, 0, 0:Q], sc(3), "relu", R_ENGS[0])
                fused(R24[:, :Q], st[p]["Mx"][:, 1, 0:Q], sc(4), "relu", R_ENGS[1])
                fused(R13[:, Q:], st[p]["My"][:, 0, 0:Q], sc(5), "relu", R_ENGS[2])
                fused(R24[:, Q:], st[p]["My"][:, 1, 0:Q], sc(6), "relu", R_ENGS[3])
                st[p]["R13"], st[p]["R24"] = R13, R24
                st[p]["Mc"] = mm_round(p, [4, 5])      # pcx, pcy
                st[p]["Mw"] = mm_round(p, [6, 7])      # pw, ph
            for p in range(PAIRS_PER_CORE):
                def sc(k, p=p):
                    return trt[:, p * NTROW + k:p * NTROW + k + 1]
                S = sb.tile([128, 2 * Q], DT, tag=f"S_{p}")
                (nc.vector if p % 2 == 0 else nc.gpsimd).tensor_tensor(
                    out=S[:], in0=st[p]["R13"][:], in1=st[p]["R24"][:], op=op.add)
                st[p]["S"] = S
                B12 = sb.tile([128, 2 * Q], DT, tag=f"B12_{p}")
                B34 = sb.tile([128, 2 * Q], DT, tag=f"B34_{p}")
                fused(B12[:, :Q], st[p]["Mc"][:, 0, 0:Q], sc(7), "abs", B_ENGS[0])
                fused(B12[:, Q:], st[p]["Mc"][:, 1, 0:Q], sc(8), "abs", B_ENGS[1])
                fused(B34[:, :Q], st[p]["Mw"][:, 0, 0:Q], sc(9), "abs", B_ENGS[2])
                fused(B34[:, Q:], st[p]["Mw"][:, 1, 0:Q], sc(10), "abs", B_ENGS[3])
                st[p]["B12"], st[p]["B34"] = B12, B34
            for p in range(PAIRS_PER_CORE):
                def sc(k, p=p):
                    return trt[:, p * NTROW + k:p * NTROW + k + 1]
                tt_eng = getattr(nc, TT_ENG)
                ve = nc.vector if p % 2 == 0 else nc.gpsimd   # alternate pairs across engines
                S = st[p]["S"]
                NW = sb.tile([128, 2 * Q], DT, tag=f"NW_{p}")
                nc.vector.tensor_scalar(out=NW[:, :Q], in0=S[:, :Q], scalar1=sc(0), scalar2=0.0,
                                        op0=op.subtract, op1=op.min)
                nc.vector.tensor_scalar(out=NW[:, Q:], in0=S[:, Q:], scalar1=sc(1), scalar2=0.0,
                                        op0=op.subtract, op1=op.min)
                WE = sb.tile([128, 2 * Q], DT, tag=f"WE_{p}")
                getattr(nc, WE_ENG).tensor_tensor(
                    out=WE[:].rearrange("p (a b) -> p a b", b=Q),
                    in0=S[:].rearrange("p (a b) -> p a b", b=Q),
                    in1=st[p]["Mw"][:, :, 0:Q], op=op.add)
                LB = sb.tile([128, 2 * Q], DT, tag=f"LB_{p}")
                tt_eng.tensor_tensor(out=LB[:], in0=st[p]["B12"][:], in1=st[p]["B34"][:], op=op.add)
                st[p]["Ml"] = mm_round(p, [8, 9])      # area1, f
                # area1|f to SBUF via ACT so tail ops can run off-PSUM on any engine
                FA = sb.tile([128, 2 * Q], DT, tag=f"FA_{p}")
                if p % 2 == 0:
                    nc.vector.tensor_copy(FA[:].rearrange("p (a b) -> p a b", b=Q),
                                          st[p]["Ml"][:, :, 0:Q])
                else:
                    nc.scalar.copy(out=FA[:].rearrange("p (a b) -> p a b", b=Q),
                                   in_=st[p]["Ml"][:, :, 0:Q])
                T1 = sb.tile([128, 2 * Q], DT, tag=f"T1_{p}")   # [inter | -union]
                ve.tensor_tensor(out=T1[:, :Q], in0=NW[:, :Q], in1=NW[:, Q:], op=op.mult)
                nc.vector.scalar_tensor_tensor(out=T1[:, Q:], in0=T1[:, :Q], scalar=sc(2),
                                               in1=FA[:, :Q], op0=op.subtract, op1=op.subtract)
                ENC = sb.tile([128, Q], DT, tag=f"ENC_{p}")
                ve.tensor_tensor(out=ENC[:], in0=WE[:, :Q], in1=WE[:, Q:], op=op.mult)
                IU = sb.tile([128, 2 * Q], DT, tag=f"IU_{p}")   # [-iou | -ue]
                RC = sb.tile([128, 2 * Q], DT, tag=f"RC_{p}")   # [1/-union | 1/enc]
                nc.vector.reciprocal(out=RC[:, :Q], in_=T1[:, Q:])
                nc.vector.reciprocal(out=RC[:, Q:], in_=ENC[:])
                nc.vector.tensor_tensor(out=IU[:, :Q], in0=T1[:, :Q], in1=RC[:, :Q], op=op.mult)
                nc.gpsimd.tensor_tensor(out=IU[:, Q:], in0=T1[:, Q:], in1=RC[:, Q:], op=op.mult)
                P1 = sb.tile([128, Q], DT, tag=f"P1_{p}")
                ve.tensor_tensor(out=P1[:], in0=IU[:, :Q], in1=IU[:, Q:], op=op.add)
                P2 = sb.tile([128, Q], DT, tag=f"P2_{p}")
                tt_eng.tensor_tensor(out=P2[:], in0=LB[:, :Q], in1=LB[:, Q:], op=op.add)
                P3 = sb.tile([128, Q], DT, tag=f"P3_{p}")
                ve.tensor_tensor(out=P3[:], in0=P2[:], in1=FA[:, Q:], op=op.add)
                OUT = sb.tile([128, Q], DT, tag=f"OUT_{p}")
                tt_eng.tensor_tensor(out=OUT[:], in0=P3[:], in1=P1[:], op=op.add)
                (nc.sync if p % 2 == 0 else nc.scalar).dma_start(out=cost_o[p], in_=OUT[:])

    _split_wide_waits(nc, mybir)
    return nc


def _lsa(cost):
    # Hungarian (shortest augmenting path), identical algorithm to reference.
    cost = np.asarray(cost, dtype=np.float64)
    n, m = cost.shape
    u = np.zeros(n + 1)
    v = np.zeros(m + 1)
    p = np.zeros(m + 1, dtype=np.int64)
    way = np.zeros(m + 1, dtype=np.int64)
    for i in range(1, n + 1):
        p[0] = i
        j0 = 0
        minv = np.full(m + 1, np.inf)
        used = np.zeros(m + 1, dtype=bool)
        while True:
            used[j0] = True
            i0 = p[j0]
            cur = cost[i0 - 1, :] - u[i0] - v[1:]
            free = ~used[1:]
            upd = free & (cur < minv[1:])
            minv[1:][upd] = cur[upd]
            way[1:][upd] = j0
            cand = np.where(free, minv[1:], np.inf)
            j1 = int(np.argmin(cand)) + 1
            delta = cand[j1 - 1]
            u[p[used]] += delta
            v[used] -= delta
            minv[~used] -= delta
            j0 = j1
            if p[j0] == 0:
                break
        while j0:
            j1 = way[j0]
            p[j0] = p[j1]
            j0 = j1
    ans = np.zeros(n, dtype=np.int64)
    for j in range(1, m + 1):
        if p[j] > 0:
            ans[p[j] - 1] = j - 1
    return ans


def _host_prep(logits, pred_bbox, target_bbox):
    import ml_dtypes
    logits = np.ascontiguousarray(logits, np.float32)
    pb = np.ascontiguousarray(pred_bbox, np.float32)
    tb = np.ascontiguousarray(target_bbox, np.float32)

    pcx, pcy, pw, ph = pb[..., 0], pb[..., 1], pb[..., 2], pb[..., 3]
    px1, py1 = pcx - 0.5 * pw, pcy - 0.5 * ph
    px2, py2 = pcx + 0.5 * pw, pcy + 0.5 * ph
    area1 = pw * ph
    dl = (logits[..., 1] - logits[..., 0]).astype(np.float64)
    f = (1.0 / (1.0 + np.exp(-dl))).astype(np.float32)   # 1 - p0 = sigmoid(l1-l0)
    # [B, 10, Q], quantity-major
    qr_all = np.stack([px1, -px2, py1, -py2, pcx, pcy, pw, ph, area1, f], axis=1)

    tcx, tcy, tw, th = tb[..., 0], tb[..., 1], tb[..., 2], tb[..., 3]
    tx1, ty1 = tcx - 0.5 * tw, tcy - 0.5 * th
    tx2, ty2 = tcx + 0.5 * tw, tcy + 0.5 * th
    area2 = tw * th
    # [B, T, 11]
    tr_all = np.stack([tw, th, area2, -tx1, tx2, -ty1, ty2, -tcx, -tcy, -tw, -th],
                      axis=-1)

    in_maps = []
    for c in range(N_CORES):
        i0 = c * IMGS_PER_CORE
        # qrows: [pair, group(3) x img(2), 4*Q] bf16, groups of quantities
        qc4 = qr_all[i0:i0 + IMGS_PER_CORE].reshape(PAIRS_PER_CORE, 2, 10, Q)
        # pre-round to bf16 so the A-B row is an exact difference of bf16 values
        qc4 = qc4.astype(ml_dtypes.bfloat16).astype(np.float32)
        qc = np.zeros((PAIRS_PER_CORE, 96, 4 * Q), np.float32)
        for g, ks in enumerate(([0, 1, 2, 3], [4, 5, 6], [7, 8, 9])):
            for j, k in enumerate(ks):
                # row0 = imgA - imgB (selected on partitions 0-63), row1 = imgB
                qc[:, g * 32 + 0, j * Q:(j + 1) * Q] = qc4[:, 0, k, :] - qc4[:, 1, k, :]
                qc[:, g * 32 + 1, j * Q:(j + 1) * Q] = qc4[:, 1, k, :]
        # trows: [128 partitions, pair*11]
        tc_ = tr_all[i0:i0 + IMGS_PER_CORE].reshape(PAIRS_PER_CORE, 128, 11)
        tc_ = tc_.transpose(1, 0, 2).reshape(128, PAIRS_PER_CORE * 11)
        in_maps.append({
            "qrows": np.ascontiguousarray(qc).astype(ml_dtypes.bfloat16),
            "trows": np.ascontiguousarray(tc_),
        })
    return in_maps


def _finalize(logits, pred_bbox, target_bbox, target_labels, src):
    labels = np.asarray(target_labels).astype(np.int64)
    lg = np.asarray(logits, np.float64)
    pb = np.asarray(pred_bbox, np.float64)
    tb = np.asarray(target_bbox, np.float64)
    bidx = np.arange(B)[:, None]

    # CE pieces (exact, host): nlpk = -logp_k
    dl = lg[..., 1] - lg[..., 0]
    nlp1 = np.logaddexp(0.0, -dl)       # -logp1 = softplus(l0-l1)
    nlp0 = np.logaddexp(0.0, dl)        # -logp0 = softplus(l1-l0)
    g = nlp0 - CLS_SCALE * nlp1         # matched-query correction (labels are 0)
    A = nlp1.sum()
    w = np.ones(C); w[-1] = CLS_SCALE
    wt_sum = CLS_SCALE * (B * Q) + np.sum(w[labels] - CLS_SCALE)
    ce = (CLS_SCALE * A + g[bidx, src].sum()) / wt_sum

    mp = pb[bidx, src].reshape(-1, 4)
    mt = tb.reshape(-1, 4)
    nb = B * T
    l1 = np.abs(mp - mt).sum() / nb

    def corners(x):
        cx, cy, ww, hh = x[:, 0], x[:, 1], x[:, 2], x[:, 3]
        return np.stack([cx - .5 * ww, cy - .5 * hh, cx + .5 * ww, cy + .5 * hh], -1)

    c1, c2 = corners(mp), corners(mt)
    a1 = (c1[:, 2] - c1[:, 0]) * (c1[:, 3] - c1[:, 1])
    a2 = (c2[:, 2] - c2[:, 0]) * (c2[:, 3] - c2[:, 1])
    lt = np.maximum(c1[:, :2], c2[:, :2]); rb = np.minimum(c1[:, 2:], c2[:, 2:])
    wh = np.clip(rb - lt, 0, None); inter = wh[:, 0] * wh[:, 1]
    union = a1 + a2 - inter
    iou = inter / union
    lte = np.minimum(c1[:, :2], c2[:, :2]); rbe = np.maximum(c1[:, 2:], c2[:, 2:])
    whe = np.clip(rbe - lte, 0, None); encl = whe[:, 0] * whe[:, 1]
    giou = iou - (encl - union) / encl
    lgi = (1.0 - giou).sum() / nb
    return ce + BBOX_SCALE * l1 + GIOU_SCALE * lgi


def kernel(logits, pred_bbox, target_bbox, target_labels):
    import os
    os.environ["BASS_NEVER_TRACE"] = "1"   # no NTFF hook in this container
    from concourse.bass_utils import run_bass_kernel_spmd

    if "nc" not in _CACHE:
        _CACHE["nc"] = _build_program()
    nc = _CACHE["nc"]

    in_maps = _host_prep(logits, pred_bbox, target_bbox)
    res = run_bass_kernel_spmd(nc, in_maps, core_ids=list(range(N_CORES)))
    _CACHE["last_res"] = res

    cost_T = np.zeros((B, T, Q), np.float32)   # [img, target, query]
    for c in range(N_CORES):
        cb = np.asarray(res.results[c]["cost"]).astype(np.float32).reshape(PAIRS_PER_CORE, 2, 64, Q)
        i0 = c * IMGS_PER_CORE
        for p in range(PAIRS_PER_CORE):
            cost_T[i0 + 2 * p] = cb[p, 0]
            cost_T[i0 + 2 * p + 1] = cb[p, 1]

    src = np.zeros((B, T), np.int64)
    for i in range(B):
        src[i] = _lsa(cost_T[i])

    total = _finalize(logits, pred_bbox, target_bbox, target_labels, src)
    return np.float32(total)



# revision 41
# speedup vs baseline: 1.1319x; 1.1319x over previous
"""DETR loss (cost matrix + Hungarian matching + losses) on 8 Trainium2 cores.

Sharding: data-parallel over batch. Each core handles 4 images as 2 pairs of 2
images packed into 128 SBUF partitions (2 images x 64 targets). The device
computes, per image, the [T=64, Q=300] matching-cost block (L1 cdist + class
cost + pairwise GIoU cost) scaled by 2; host runs the (inherently serial)
Hungarian assignment exactly as the reference does, and assembles the scalar
loss from matched pairs in f64.

Device design (CoreSim cost-model driven, walrus-legal):
- Broadcast matmuls: out[pt,q] = indDelta[pt]*(xA-xB)[q] + xB[q] + bias(pt),
  with per-target biases folded in via extra lhsT rows against constant-one
  rhs rows (bias split hi+lo bf16). 8 PSUM slots per pair:
  2dx, dw, 2dy, dh, pw+tw, ph+th, 2f, 4(area1+area2).
- Interval identity halves the slot count: overlap_w = (pw+tw)/2-max(|dx|,|dw|/2)
  and enclose_w = (pw+tw)/2+max(|dx|,|dw|/2), so the same |2dx|,|dw| absolutes
  feed the intersection, enclosure AND the L1 cost (|dx|+|dy| reuse).
- PSUM->SBUF crossings are the scarce resource (Pool/GPSIMD cannot touch PSUM,
  at most one PSUM operand per DVE op): ACT evacuates [|2dx|,|dw|], [|2dy|,|dh|]
  (Abs) and [Mw,2f,AS] (Relu, positive data) as wide multi-slot activations.
- GIoU tail via one split reciprocal (1/enc early, 1/union late) + flat
  tensor_tensor ops spread across Pool/DVE per the ENG table.
- Inputs land as TALL DMAs ([128, narrow]) since DMA cost is per-partition
  bytes; lhsT rides as [128, 384] with 4-row groups at bases 0/32/64.
"""
import numpy as np

B, Q, T, C = 32, 300, 64, 2
N_CORES = 8
IMGS_PER_CORE = B // N_CORES          # 4
PAIRS_PER_CORE = IMGS_PER_CORE // 2   # 2
CLS_SCALE = 0.1
BBOX_SCALE = 5.0
GIOU_SCALE = 2.0

_CACHE = {}

# per-op engine assignment knobs (value per pair), tuned via CoreSim.
# Crossings: cXA/cXB (ACT Abs) and cMF (ACT Relu) fixed on ACT; RC/NW/OUT on DVE.
ENG = {
    "XA": ["scalar", "scalar"],
    "XB": ["scalar", "scalar"],      # |2dy|,|dh| crossing: ACT abs / DVE ts-abs
    "H": ["vector", "vector"],       # Hx, Hy max (tt)
    "V": ["vector", "vector"],       # H - Mw [600] tt
    "W": ["vector", "vector"],       # H + Mw [600] tt
    "NW": ["vector", "vector"],      # min(V,0) [600] ts
    "IN": ["gpsimd", "gpsimd"],
    "EN": ["gpsimd", "gpsimd"],
    "U": ["gpsimd", "gpsimd"],
    "RC": ["vector", "vector"],
    "D1": ["gpsimd", "gpsimd"],
    "D2": ["gpsimd", "gpsimd"],
    "P1": ["gpsimd", "gpsimd"],
    "L2X": ["vector", "vector"],     # XA + XB [600] tt
    "T1": ["gpsimd", "gpsimd"],
    "L2F": ["gpsimd", "gpsimd"],
    "OUT": ["vector", "vector"],     # stt (T1*2)+L2F
}


def _split_wide_waits(nc, mybir, max_waits=1):
    """Walrus rejects instructions carrying >1 sem-wait; hoist extra waits
    onto NoOp carriers inserted just before (same engine, in-order)."""
    n_new = 0
    for bb in nc.main_func.blocks:
        insts = bb.instructions
        i = 0
        while i < len(insts):
            ins = insts[i]
            si = ins.sync_info
            if (
                si is not None
                and si.on_wait is not None
                and len(si.on_wait) > max_waits
            ):
                waits = list(si.on_wait)
                si.on_wait = waits[:max_waits]
                extra = waits[max_waits:]
                for j in range(0, len(extra), max_waits):
                    nd = mybir.InstNoOp(name=f"{ins.name}-xw{n_new}", ins=[], outs=[])
                    nd.engine = ins.engine
                    nd.sync_info = mybir.SyncInfo(
                        on_wait=extra[j : j + max_waits], on_update=[]
                    )
                    nc.register_instruction(nd, overwrite=True)
                    insts.insert(i, nd)
                    n_new += 1
                    i += 1
            i += 1
    return n_new


def _build_program():
    import concourse.bass as bass
    import concourse.mybir as mybir
    from concourse.tile import TileContext

    f32 = mybir.dt.float32
    bf16 = mybir.dt.bfloat16
    op = mybir.AluOpType
    AF = mybir.ActivationFunctionType
    NP = PAIRS_PER_CORE

    nc = bass.Bass()
    rh = nc.declare_dram_parameter("rh", [NP, 128, 900], bf16, isOutput=False)
    lt = nc.declare_dram_parameter("lt", [NP, 128, 384], bf16, isOutput=False)
    f2b = nc.declare_dram_parameter("f2b", [NP, 128, Q], bf16, isOutput=False)
    cost_o = nc.declare_dram_parameter("cost", [NP, 128, Q], bf16, isOutput=True)

    with TileContext(nc) as tc:
        with (
            nc.allow_low_precision(reason="bf16 cost pipeline; assignment-tolerant"),
            tc.tile_pool(name="const", bufs=1) as cpool,
            tc.tile_pool(name="sb", bufs=2) as sb,
            tc.tile_pool(name="ps", bufs=2, space="PSUM") as ps,
        ):
            # tiny tile for ACT table warm; init on DVE (fast, idle at t0)
            warm = cpool.tile([1, 2], bf16)
            nc.vector.memset(warm[:], 0.25)
            zsc = cpool.tile([128, 1], f32)
            nc.vector.memset(zsc[:], 0.0)

            rhts, ltts = [], []
            for p in range(NP):
                rhts.append(sb.tile([128, 900], bf16, name=f"rht{p}", tag=f"rh{p}"))
                ltts.append(sb.tile([128, 384], bf16, name=f"ltt{p}", tag=f"lt{p}"))

            # DMA triggers, urgency-ordered per engine.
            nc.scalar.dma_start(out=rhts[0][:, 0:600], in_=rh[0][:, 0:600])
            nc.sync.dma_start(out=ltts[0][:], in_=lt[0][:])
            nc.gpsimd.dma_start(out=rhts[0][:, 600:900], in_=rh[0][:, 600:900])
            nc.gpsimd.dma_start(out=rhts[1][:, 0:600], in_=rh[1][:, 0:600])
            nc.sync.dma_start(out=ltts[1][:], in_=lt[1][:])
            nc.sync.dma_start(out=rhts[1][:, 600:900], in_=rh[1][:, 600:900])
            f2ts = []
            for p in range(NP):
                f2ts.append(sb.tile([128, Q], bf16, name=f"f2t{p}", tag=f"f2_{p}"))
                nc.sync.dma_start(out=f2ts[p][:], in_=f2b[p])

            # warm the act table (Relu+Abs share a set)
            wo = cpool.tile([1, 2], bf16)
            nc.scalar.activation(wo[:], warm[:], AF.Relu)
            nc.scalar.activation(wo[:], warm[:], AF.Abs)

            def eng(key, p):
                return getattr(nc, ENG[key][p])

            # slot j -> (partition base 32*(j%3), col group j//3)
            # j: 0 t2dx(2pcx-2tcx) 1 dw(pw-tw) 2 t2dy | 3 dh 4 Mwx(pw+tw) 5 Mwy(ph+th)
            #    6 f2(2f) 7 AS4(4area1+4area2)
            def mm(p, j, pst, slot):
                b = 32 * (j % 3)
                cg = j // 3
                nc.tensor.matmul(
                    pst[:, 512 * slot:512 * slot + Q],
                    lhsT=ltts[p][b:b + 4, 128 * cg:128 * cg + 128],
                    rhs=rhts[p][b:b + 4, 300 * cg:300 * cg + 300],
                    start=True, stop=True,
                )

            def v3(ap, w=300):
                return ap.rearrange("p (s k) -> p s k", k=512)[:, :, 0:w]

            def vs(ap, w=300):
                return ap.rearrange("p (s k) -> p s k", k=w)

            st = [dict() for _ in range(NP)]

            def stage_mm_X(p):
                mXA = ps.tile([128, 1024], f32, name=f"mXA_{p}", tag="m2a", bufs=1)
                mXB = ps.tile([128, 1024], f32, name=f"mXB_{p}", tag="m2b", bufs=1)
                mm(p, 0, mXA, 0)   # 2dx
                mm(p, 1, mXA, 1)   # dw
                mm(p, 2, mXB, 0)   # 2dy
                mm(p, 3, mXB, 1)   # dh
                st[p]["mXA"], st[p]["mXB"] = mXA, mXB

            def stage_mm_MF(p):
                mMF = ps.tile([128, 1536], f32, name=f"mMF_{p}", tag="m3", bufs=1)
                mm(p, 4, mMF, 0)   # Mwx
                mm(p, 5, mMF, 1)   # Mwy
                mm(p, 7, mMF, 2)   # AS4
                st[p]["mMF"] = mMF

            def stage_XABS(p):
                s = st[p]
                XA = sb.tile([128, 600], bf16, name=f"XA_{p}", tag=f"XA_{p}")
                XB = sb.tile([128, 600], bf16, name=f"XB_{p}", tag=f"XB_{p}")
                if ENG["XA"][p] == "scalar":
                    nc.scalar.activation(vs(XA[:]), v3(s["mXA"][:]), AF.Abs)
                else:
                    nc.vector.tensor_scalar(out=XA[:, 0:300], in0=s["mXA"][:, 0:300],
                                            scalar1=zsc[:, 0:1], scalar2=0.0,
                                            op0=op.add, op1=op.abs_max)
                    nc.vector.tensor_scalar(out=XA[:, 300:600],
                                            in0=s["mXA"][:, 512:812],
                                            scalar1=zsc[:, 0:1], scalar2=0.0,
                                            op0=op.add, op1=op.abs_max)
                nc.scalar.activation(vs(XB[:]), v3(s["mXB"][:]), AF.Abs)
                s["XA"], s["XB"] = XA, XB    # [|2dx|,|dw|], [|2dy|,|dh|]

            def stage_MF(p):
                s = st[p]
                MF = sb.tile([128, 900], bf16, name=f"MF_{p}", tag=f"MF_{p}")
                nc.scalar.activation(vs(MF[:]), v3(s["mMF"][:]), AF.Relu)
                s["MF"] = MF    # [Mwx, Mwy, 4(a1+a2)]

            def stage_H(p):
                s = st[p]
                H = sb.tile([128, 600], bf16, name=f"H_{p}", tag=f"H_{p}")
                e = eng("H", p)
                e.tensor_tensor(out=H[:, 0:300], in0=s["XA"][:, 0:300],
                                in1=s["XA"][:, 300:600], op=op.max)
                e.tensor_tensor(out=H[:, 300:600], in0=s["XB"][:, 0:300],
                                in1=s["XB"][:, 300:600], op=op.max)
                s["H"] = H      # [Hx, Hy]

            def stage_VW(p):
                s = st[p]
                V = sb.tile([128, 600], bf16, name=f"V_{p}", tag=f"V_{p}")
                eng("V", p).tensor_tensor(out=V[:], in0=s["H"][:],
                                          in1=s["MF"][:, 0:600], op=op.subtract)
                W = sb.tile([128, 600], bf16, name=f"W_{p}", tag=f"W_{p}")
                eng("W", p).tensor_tensor(out=W[:], in0=s["H"][:],
                                          in1=s["MF"][:, 0:600], op=op.add)
                NW = sb.tile([128, 600], bf16, name=f"NW_{p}", tag=f"NW_{p}")
                eng("NW", p).tensor_scalar(out=NW[:], in0=V[:], scalar1=0.0,
                                           scalar2=None, op0=op.min)
                s["W"], s["NW"] = W, NW

            def stage_IE(p):
                s = st[p]
                IN = sb.tile([128, 300], bf16, name=f"IN_{p}", tag=f"IN_{p}")
                eng("IN", p).tensor_tensor(out=IN[:], in0=s["NW"][:, 0:300],
                                           in1=s["NW"][:, 300:600], op=op.mult)
                s["IN"] = IN    # 4*inter
                EN = sb.tile([128, 300], bf16, name=f"EN_{p}", tag=f"EN_{p}")
                eng("EN", p).tensor_tensor(out=EN[:], in0=s["W"][:, 0:300],
                                           in1=s["W"][:, 300:600], op=op.mult)
                RCE = sb.tile([128, 300], bf16, name=f"RCE_{p}", tag=f"RCE_{p}")
                nc.vector.reciprocal(out=RCE[:], in_=EN[:])
                s["RCE"] = RCE
                UU = sb.tile([128, 300], bf16, name=f"UU_{p}", tag=f"UU_{p}")
                eng("U", p).tensor_tensor(out=UU[:], in0=s["MF"][:, 600:900],
                                          in1=IN[:], op=op.subtract)
                s["UU"] = UU

            def stage_GI(p):
                s = st[p]
                RCU = sb.tile([128, 300], bf16, name=f"RCU_{p}", tag=f"RCU_{p}")
                nc.vector.reciprocal(out=RCU[:], in_=s["UU"][:])
                D2 = sb.tile([128, 300], bf16, name=f"D2_{p}", tag=f"D2_{p}")
                eng("D2", p).tensor_tensor(out=D2[:], in0=s["UU"][:],
                                           in1=s["RCE"][:], op=op.mult)
                T1 = sb.tile([128, 300], bf16, name=f"T1_{p}", tag=f"T1_{p}")
                eng("T1", p).tensor_tensor(out=T1[:], in0=s["L2X"][:, 300:600],
                                           in1=D2[:], op=op.subtract)
                D1 = sb.tile([128, 300], bf16, name=f"D1_{p}", tag=f"D1_{p}")
                eng("D1", p).tensor_tensor(out=D1[:], in0=s["IN"][:],
                                           in1=RCU[:], op=op.mult)
                T2 = sb.tile([128, 300], bf16, name=f"T2_{p}", tag=f"T2_{p}")
                eng("P1", p).tensor_tensor(out=T2[:], in0=T1[:], in1=D1[:],
                                           op=op.subtract)
                s["T2"] = T2    # X34 - iou - union/enc

            def stage_L(p):
                s = st[p]
                L2X = sb.tile([128, 600], bf16, name=f"L2X_{p}", tag=f"L2X_{p}")
                eng("L2X", p).tensor_tensor(out=L2X[:], in0=s["XA"][:],
                                            in1=s["XB"][:], op=op.add)
                s["L2X"] = L2X  # [2(|dx|+|dy|), |dw|+|dh|]

            def stage_L2F(p):
                s = st[p]
                L2F = sb.tile([128, 300], bf16, name=f"L2F_{p}", tag=f"L2F_{p}")
                eng("L2F", p).tensor_tensor(out=L2F[:], in0=s["L2X"][:, 0:300],
                                            in1=f2ts[p][:], op=op.add)
                s["L2F"] = L2F

            def stage_OUT(p):
                s = st[p]
                OUT = sb.tile([128, 300], bf16, name=f"OUT_{p}", tag=f"OUT_{p}")
                eng("OUT", p).scalar_tensor_tensor(out=OUT[:], in0=s["T2"][:],
                                                   scalar=2.0, op0=op.mult,
                                                   op1=op.add, in1=s["L2F"][:])
                nc.sync.dma_start(out=cost_o[p], in_=OUT[:])

            # global issue order
            stage_mm_X(0)
            stage_XABS(0)
            stage_mm_MF(0)
            stage_H(0)
            stage_MF(0)
            stage_mm_X(1)
            stage_L(0)
            stage_VW(0)
            stage_XABS(1)
            stage_IE(0)
            stage_mm_MF(1)
            stage_H(1)
            stage_L2F(0)
            stage_GI(0)
            stage_MF(1)
            stage_L(1)
            stage_VW(1)
            stage_OUT(0)
            stage_IE(1)
            stage_L2F(1)
            stage_GI(1)
            stage_OUT(1)

    _split_wide_waits(nc, mybir)
    return nc


def _lsa(cost):
    # Hungarian (shortest augmenting path), identical algorithm to reference.
    cost = np.asarray(cost, dtype=np.float64)
    n, m = cost.shape
    u = np.zeros(n + 1)
    v = np.zeros(m + 1)
    p = np.zeros(m + 1, dtype=np.int64)
    way = np.zeros(m + 1, dtype=np.int64)
    for i in range(1, n + 1):
        p[0] = i
        j0 = 0
        minv = np.full(m + 1, np.inf)
        used = np.zeros(m + 1, dtype=bool)
        while True:
            used[j0] = True
            i0 = p[j0]
            cur = cost[i0 - 1, :] - u[i0] - v[1:]
            free = ~used[1:]
            upd = free & (cur < minv[1:])
            minv[1:][upd] = cur[upd]
            way[1:][upd] = j0
            cand = np.where(free, minv[1:], np.inf)
            j1 = int(np.argmin(cand)) + 1
            delta = cand[j1 - 1]
            u[p[used]] += delta
            v[used] -= delta
            minv[~used] -= delta
            j0 = j1
            if p[j0] == 0:
                break
        while j0:
            j1 = way[j0]
            p[j0] = p[j1]
            j0 = j1
    ans = np.zeros(n, dtype=np.int64)
    for j in range(1, m + 1):
        if p[j] > 0:
            ans[p[j] - 1] = j - 1
    return ans


def _host_prep(logits, pred_bbox, target_bbox):
    import ml_dtypes
    bf = ml_dtypes.bfloat16

    def rnd(x):
        return np.asarray(x, np.float32).astype(bf).astype(np.float32)

    logits = np.ascontiguousarray(logits, np.float32)
    pb = np.ascontiguousarray(pred_bbox, np.float32)
    tb = np.ascontiguousarray(target_bbox, np.float32)

    pcx, pcy, pw, ph = pb[..., 0], pb[..., 1], pb[..., 2], pb[..., 3]
    area1 = pw * ph
    dl = (logits[..., 1] - logits[..., 0]).astype(np.float64)
    f = (1.0 / (1.0 + np.exp(-dl))).astype(np.float32)   # 1 - p0 = sigmoid(l1-l0)

    # per-query quantity rows [B, 8, Q], bf16-rounded
    # slots: 2pcx, pw, 2pcy, ph, pw, ph, 2f, 4*area1
    qrows = np.stack([
        rnd(2 * pcx), rnd(pw), rnd(2 * pcy), rnd(ph),
        rnd(pw), rnd(ph), rnd(2 * f), rnd(4 * area1),
    ], axis=1)

    tcx, tcy, tw, th = tb[..., 0], tb[..., 1], tb[..., 2], tb[..., 3]
    area2 = tw * th
    zeros = np.zeros_like(tw)
    # per-target biases [B, 8, T]
    brows = np.stack([
        -2 * tcx, -tw, -2 * tcy, -th,
        tw, th, zeros, 4 * area2,
    ], axis=1)

    ind = (np.arange(128) < 64).astype(np.float32)  # 1 for image-A partitions

    in_maps = []
    for c in range(N_CORES):
        i0 = c * IMGS_PER_CORE
        rh = np.zeros((PAIRS_PER_CORE, 128, 900), np.float32)
        ltm = np.zeros((PAIRS_PER_CORE, 128, 384), np.float32)
        for p in range(PAIRS_PER_CORE):
            iA, iB = i0 + 2 * p, i0 + 2 * p + 1
            bias = np.concatenate([brows[iA], brows[iB]], axis=1)  # [8, 128]
            bias_hi = bias.astype(bf).astype(np.float32)
            bias_lo = (bias - bias_hi).astype(bf).astype(np.float32)
            for j in (0, 1, 2, 3, 4, 5, 7):
                b = 32 * (j % 3)
                cg = j // 3
                cs, ce = 300 * cg, 300 * cg + 300
                rh[p, b + 0, cs:ce] = qrows[iA, j] - qrows[iB, j]
                rh[p, b + 1, cs:ce] = qrows[iB, j]
                rh[p, b + 2, cs:ce] = 1.0
                rh[p, b + 3, cs:ce] = 1.0
                ls, le = 128 * cg, 128 * cg + 128
                ltm[p, b + 0, ls:le] = ind
                ltm[p, b + 1, ls:le] = 1.0
                ltm[p, b + 2, ls:le] = bias_hi[j]
                ltm[p, b + 3, ls:le] = bias_lo[j]
        f2m = np.zeros((PAIRS_PER_CORE, 128, Q), np.float32)
        for p in range(PAIRS_PER_CORE):
            iA, iB = i0 + 2 * p, i0 + 2 * p + 1
            f2m[p, 0:64, :] = qrows[iA, 6]
            f2m[p, 64:128, :] = qrows[iB, 6]
        in_maps.append({
            "rh": rh.astype(bf),
            "lt": ltm.astype(bf),
            "f2b": f2m.astype(bf),
        })
    return in_maps


def _finalize(logits, pred_bbox, target_bbox, target_labels, src):
    labels = np.asarray(target_labels).astype(np.int64)
    lg = np.asarray(logits, np.float64)
    pb = np.asarray(pred_bbox, np.float64)
    tb = np.asarray(target_bbox, np.float64)
    bidx = np.arange(B)[:, None]

    # CE pieces (exact, host): nlpk = -logp_k
    dl = lg[..., 1] - lg[..., 0]
    nlp1 = np.logaddexp(0.0, -dl)       # -logp1 = softplus(l0-l1)
    nlp0 = np.logaddexp(0.0, dl)        # -logp0 = softplus(l1-l0)
    g = nlp0 - CLS_SCALE * nlp1         # matched-query correction (labels are 0)
    A = nlp1.sum()
    w = np.ones(C); w[-1] = CLS_SCALE
    wt_sum = CLS_SCALE * (B * Q) + np.sum(w[labels] - CLS_SCALE)
    ce = (CLS_SCALE * A + g[bidx, src].sum()) / wt_sum

    mp = pb[bidx, src].reshape(-1, 4)
    mt = tb.reshape(-1, 4)
    nb = B * T
    l1 = np.abs(mp - mt).sum() / nb

    def corners(x):
        cx, cy, ww, hh = x[:, 0], x[:, 1], x[:, 2], x[:, 3]
        return np.stack([cx - .5 * ww, cy - .5 * hh, cx + .5 * ww, cy + .5 * hh], -1)

    c1, c2 = corners(mp), corners(mt)
    a1 = (c1[:, 2] - c1[:, 0]) * (c1[:, 3] - c1[:, 1])
    a2 = (c2[:, 2] - c2[:, 0]) * (c2[:, 3] - c2[:, 1])
    lt = np.maximum(c1[:, :2], c2[:, :2]); rb = np.minimum(c1[:, 2:], c2[:, 2:])
    wh = np.clip(rb - lt, 0, None); inter = wh[:, 0] * wh[:, 1]
    union = a1 + a2 - inter
    iou = inter / union
    lte = np.minimum(c1[:, :2], c2[:, :2]); rbe = np.maximum(c1[:, 2:], c2[:, 2:])
    whe = np.clip(rbe - lte, 0, None); encl = whe[:, 0] * whe[:, 1]
    giou = iou - (encl - union) / encl
    lgi = (1.0 - giou).sum() / nb
    return ce + BBOX_SCALE * l1 + GIOU_SCALE * lgi


def kernel(logits, pred_bbox, target_bbox, target_labels):
    import os
    os.environ["BASS_NEVER_TRACE"] = "1"   # no NTFF hook in this container
    from concourse.bass_utils import run_bass_kernel_spmd

    if "nc" not in _CACHE:
        _CACHE["nc"] = _build_program()
    nc = _CACHE["nc"]

    in_maps = _host_prep(logits, pred_bbox, target_bbox)
    res = run_bass_kernel_spmd(nc, in_maps, core_ids=list(range(N_CORES)))
    _CACHE["last_res"] = res

    cost_T = np.zeros((B, T, Q), np.float32)   # [img, target, query]
    for c in range(N_CORES):
        cb = 0.5 * np.asarray(res.results[c]["cost"]).astype(np.float32).reshape(PAIRS_PER_CORE, 2, 64, Q)
        i0 = c * IMGS_PER_CORE
        for p in range(PAIRS_PER_CORE):
            cost_T[i0 + 2 * p] = cb[p, 0]
            cost_T[i0 + 2 * p + 1] = cb[p, 1]

    src = np.zeros((B, T), np.int64)
    for i in range(B):
        src[i] = _lsa(cost_T[i])

    total = _finalize(logits, pred_bbox, target_bbox, target_labels, src)
    return np.float32(total)
